# revision 1
# baseline (speedup 1.0000x reference)
"""Trainium2 Bass kernel for nn_BoundarySeg (segment_reduce).

out[b, j, 0:H]   = sum_{i>=j} A[b, j, i] * h[b, i, :]
out[b, j, H:2H]  = h[b, j, :] * sum_{i>=j} A[b, j, i]

Shapes: A [8, 2048, 2048] f32, h [8, 2048, 256] f32 -> out [8, 2048, 512] f32.
Sharding: data-parallel over batch; core c computes batch c.

Per-core algorithm (L=2048 in 16 tiles of 128, H=256):
  - h loads once via SWDGE (gpsimd) DMA with an in-flight fp32->f32r cast
    into [128(p), 16(t), 258], plus a ones column at [.., 256] so the
    masked row-sum falls out of the main matmul as an extra column
    (f32r matmuls need an even moving dim, hence 258).
  - For each j-tile jc: DMA only the upper panel A[jc, jc:] (lower
    triangle never loaded), transpose each 128x128 block on TensorE
    through PSUM (batches of GROUP per bank), round to f32r during the
    PSUM->SBUF copy; the diagonal block is masked (keep i >= j) by the
    same copy via a tensor_tensor multiply.
  - acc[j, n] += At_block^T @ h_ext over i-tiles >= jc (f32r, N=258).
    first half = acc[:, 0:256]; second half = h[j, :] * acc[:, 256].
  - j-tiles processed in order [8..15, 0..7] (small panels first, so the
    pipeline primes quickly) and matmuls run one iteration behind the
    transposes (PE stream never blocks the next panel's transposes).
  - DMA rings: A panels on SP (sync) HWDGE, outputs on ACT (scalar)
    HWDGE, h on SWDGE - three independent issue paths.
"""

import os
import sys

import numpy as np

sys.path.insert(0, "/opt/trn_rl_repo")

import concourse.bass as bass  # noqa: E402
import concourse.bacc as bacc  # noqa: E402
import concourse.tile as tile  # noqa: E402
from concourse import mybir  # noqa: E402
from concourse.bass_utils import run_bass_kernel_spmd  # noqa: E402
from concourse.masks import make_identity, make_lower_triangular  # noqa: E402

B, L, H = 8, 2048, 256
P = 128
GROUP = 4  # 128-col transposes batched per PSUM tile / DVE copy

DT = mybir.dt.float32

# Results of the last run (exec_time_ns etc.) for the test harness.
LAST_RESULTS = None
_NC_CACHE = {}


def _build_nc(L=L, H=H, mm_dtype=mybir.dt.float32r):
    NT = L // P
    HE = H + 2  # even N for f32r; col H = ones (rowsum), col H+1 unused
    f32r = mm_dtype

    nc = bacc.Bacc(None, target_bir_lowering=False)
    a_dram = nc.dram_tensor("a", [L, L], DT, kind="ExternalInput")
    h_dram = nc.dram_tensor("h", [L, H], DT, kind="ExternalInput")
    out_dram = nc.dram_tensor("out", [L, 2 * H], DT, kind="ExternalOutput")

    half = NT // 2
    # Biggest panels first: maximizes PE work per arriving byte, and the
    # per-group chunking keeps first-chunk latency low.
    jc_order = list(range(0, NT))

    with tile.TileContext(nc) as tc:
        with (
            tc.tile_pool(name="const", bufs=1) as const_pool,
            tc.tile_pool(name="hpool", bufs=1) as h_pool,
            tc.tile_pool(name="apanel", bufs=12) as a_pool,
            tc.tile_pool(name="atT", bufs=5) as at_pool,
            tc.tile_pool(name="tp", bufs=5, space=bass.MemorySpace.PSUM) as tp_pool,
            tc.tile_pool(name="acc", bufs=2, space=bass.MemorySpace.PSUM) as acc_pool,
            tc.tile_pool(name="outsb", bufs=4) as out_pool,
            tc.tile_pool(name="small", bufs=2) as small_pool,
        ):
            identity = const_pool.tile([P, P], DT)
            make_identity(nc, identity[:])
            # Mask for the *transposed* diagonal block ([i(part), j(free)],
            # keep i >= j -> lower triangular); columns P.. multiply by 1.0.
            # Bounced through DVE so consumers depend on DVE, not Pool.
            mask_src = const_pool.tile([P, P], DT)
            make_lower_triangular(nc, mask_src[:], val=1.0, diag=True)
            cmask = const_pool.tile([P, GROUP * P], DT)
            nc.vector.tensor_copy(cmask[:, 0:P], mask_src[:])
            nc.vector.memset(cmask[:, P : GROUP * P], 1.0)

            # h: one half per HWDGE ring, emitted before the panel chunks
            # (measured better than chunks-first), staged in fp32 with the
            # ones columns, then DVE cast-copies to f32r per half.
            h_stage = h_pool.tile([P, NT, HE], DT)
            h_all = h_pool.tile([P, NT, HE], f32r)
            h_re = h_dram[:].rearrange("(t p) n -> p t n", p=P)
            nc.sync.dma_start(out=h_stage[:, 0:half, 0:H], in_=h_re[:, 0:half, :])
            nc.scalar.dma_start(out=h_stage[:, half:NT, 0:H], in_=h_re[:, half:NT, :])
            nc.vector.memset(h_stage[:, :, H:HE], 1.0)
            nc.vector.tensor_copy(h_all[:, half:NT, :], h_stage[:, half:NT, :])
            nc.vector.tensor_copy(h_all[:, 0:half, :], h_stage[:, 0:half, :])

            # Warmup transpose: absorbs the Pool->PE wait for `identity`.
            wtp = tp_pool.tile([P, GROUP * P], DT, tag="tp")
            nc.tensor.transpose(wtp[:, 0:P], identity[:], identity[:])

            def matmuls_and_store(jc, atT):
                ntiles = NT - jc
                acc = acc_pool.tile([P, HE], DT, tag="acc")
                for k in range(ntiles):
                    nc.tensor.matmul(
                        acc[:],
                        atT[:, k * P : (k + 1) * P],
                        h_all[:, jc + k, :],
                        start=(k == 0),
                        stop=(k == ntiles - 1),
                    )
                out_sb = out_pool.tile([P, 2 * H], DT, tag="outsb")
                rowsum = small_pool.tile([P, 1], DT, tag="rowsum")
                nc.scalar.copy(rowsum[:], acc[:, H : H + 1])
                nc.vector.tensor_copy(out_sb[:, 0:H], acc[:, 0:H])
                nc.scalar.activation(
                    out_sb[:, H : 2 * H],
                    h_stage[:, jc, 0:H],
                    mybir.ActivationFunctionType.Identity,
                    scale=rowsum[:],
                )
                nc.gpsimd.dma_start(out_dram[jc * P : (jc + 1) * P, :], out_sb[:])

            pending = []  # (jc, atT) whose matmuls run two iterations later
            ring = [nc.sync, nc.scalar]  # alternate chunk DMAs across HWDGE rings
            ring_i = 0
            for jc in jc_order:
                ntiles = NT - jc
                W = ntiles * P

                # Load the panel as per-GROUP chunks (256 KB each) so the
                # first chunk lands quickly even when several transfers are
                # in flight, and transpose each chunk as soon as it arrives.
                atT = at_pool.tile([P, W], f32r, tag="atT")
                for g0 in range(0, ntiles, GROUP):
                    gn = min(GROUP, ntiles - g0)
                    a_chunk = a_pool.tile([P, GROUP * P], DT, tag="apanel")
                    ring[ring_i % 2].dma_start(
                        a_chunk[:, 0 : gn * P],
                        a_dram[
                            jc * P : (jc + 1) * P,
                            (jc + g0) * P : (jc + g0 + gn) * P,
                        ],
                    )
                    ring_i += 1
                    tp = tp_pool.tile([P, GROUP * P], DT, tag="tp")
                    for k in range(gn):
                        nc.tensor.transpose(
                            tp[:, k * P : (k + 1) * P],
                            a_chunk[:, k * P : (k + 1) * P],
                            identity[:],
                        )
                    if g0 == 0:
                        nc.vector.tensor_tensor(
                            atT[:, 0 : gn * P],
                            tp[:, 0 : gn * P],
                            cmask[:, 0 : gn * P],
                            mybir.AluOpType.mult,
                        )
                    else:
                        nc.vector.tensor_copy(
                            atT[:, g0 * P : (g0 + gn) * P], tp[:, 0 : gn * P]
                        )

                pending.append((jc, atT))
                if len(pending) > 2:
                    matmuls_and_store(*pending.pop(0))

            for item in pending:
                matmuls_and_store(*item)

    nc.finalize()
    return nc


def kernel(span_adjacency, bound_hidden):
    global LAST_RESULTS
    a = np.ascontiguousarray(np.asarray(span_adjacency, dtype=np.float32))
    h = np.ascontiguousarray(np.asarray(bound_hidden, dtype=np.float32))
    assert a.shape == (B, L, L) and h.shape == (B, L, H), (a.shape, h.shape)

    key = "full"
    if key not in _NC_CACHE:
        _NC_CACHE[key] = _build_nc()
    nc = _NC_CACHE[key]

    in_maps = [{"a": a[b], "h": h[b]} for b in range(B)]
    res = run_bass_kernel_spmd(
        nc,
        in_maps,
        core_ids=list(range(B)),
        trace=bool(os.environ.get("KERNEL_TRACE")),
    )
    LAST_RESULTS = res
    out = np.stack([res.results[b]["out"] for b in range(B)], axis=0)
    return out

